# revision 6
# baseline (speedup 1.0000x reference)
"""Trainium2 Bass kernel for nn_BasisMatrixReadout (GNN message passing ->
dense symmetric block matrix readout).

Strategy (8 NeuronCores, SPMD):
  - Output M [13000, 13000] f32 sharded by node row-blocks: core k owns nodes
    [125k, 125k+125) -> rows [1625k, 1625k+1625).
  - Rows are further split into 13 row-family tensors out_x (row r = 13*rb+x)
    so scatter-call chains to different families are independent.
  - All float math on device: node_h via adjacency-count matmul (segment_sum
    == A @ m), edge messages / projections as feature-major matmuls, per-type
    block ops folded into C1 = 0.5*W_edge@cob, C2 = 0.5*W_edge@cob^T(xy),
    Ccomb = 2*(C1+C2) for self-edges; node diagonal blocks via
    D_tau = W_node@cob_node.
  - Host does integer-only schedule construction (placement lists sorted by
    (collision-wave, kind, edge-type), padded to cross-core maxima so a single
    SPMD program serves all cores); gathers use indirect DMA with index
    tensors; block scatter uses indirect DMA (13-float rows) with CCE add onto
    zero-filled outputs; collision waves live in dedicated trailing tiles so
    no two descriptors of one call target the same address.
"""
import sys

sys.path.insert(0, "/opt/trn_rl_repo")
import numpy as np

# ---------------- problem constants (hardcoded per spec) ----------------
N, E, T, P, D = 1000, 20000, 4, 10, 13
F, H, EA, KDIM = 128, 128, 16, 64
I_EDGE = D * D            # 169
I_NODE = D * (D + 1) // 2  # 91
NCORES = 8
NPC = N // NCORES         # 125 nodes per core
DUMP_SLOT = NPC * N       # dump row (row 125) slot base in out_x
KINDS = ("fwd", "rev", "self")


# ---------------- host-side integer schedule ----------------
def _build_adjacency(src, dst):
    A = np.zeros((N, N), np.float32)
    np.add.at(A, (dst, src), 1.0)
    np.add.at(A, (src, dst), 1.0)
    return A


def _core_placements(src, dst, etype, k):
    base = k * NPC
    hi = base + NPC
    pl = []
    for kind, mask in (("fwd", (src >= base) & (src < hi) & (src != dst)),
                       ("rev", (dst >= base) & (dst < hi) & (src != dst)),
                       ("self", (src == dst) & (src >= base) & (src < hi))):
        for e in np.nonzero(mask)[0]:
            if kind == "fwd":
                rb, c = src[e] - base, dst[e]
            elif kind == "rev":
                rb, c = dst[e] - base, src[e]
            else:
                rb, c = src[e] - base, src[e]
            pl.append({"kind": kind, "t": int(etype[e]), "rb": int(rb),
                       "c": int(c), "e": int(e)})
    counts = {}
    for p in pl:
        tgt = (p["rb"], p["c"])
        w = counts.get(tgt, 1 if p["c"] == base + p["rb"] else 0)
        p["wave"] = w
        counts[tgt] = w + 1
    return pl


def _build_schedule(edge_index, edge_types):
    src = edge_index[0].astype(np.int64)
    dst = edge_index[1].astype(np.int64)
    et = edge_types.astype(np.int64)
    percore = [_core_placements(src, dst, et, k) for k in range(NCORES)]

    def gkey(p):
        return (p["wave"], KINDS.index(p["kind"]), p["t"])

    sizes = {}
    for pl in percore:
        cnt = {}
        for p in pl:
            cnt[gkey(p)] = cnt.get(gkey(p), 0) + 1
        for kk, v in cnt.items():
            sizes[kk] = max(sizes.get(kk, 0), v)
    # pad groups to multiples of 32 so matmul sub-runs start 32-aligned
    for kk in sizes:
        sizes[kk] = (sizes[kk] + 31) // 32 * 32

    group_order = sorted(sizes)
    slot_of = {}
    cursor = 0
    cur_wave = 0
    for g in group_order:
        if g[0] != cur_wave:
            cursor = (cursor + 127) // 128 * 128  # wave -> tile boundary
            cur_wave = g[0]
        slot_of[g] = cursor
        cursor += sizes[g]
    n_slots = (cursor + 127) // 128 * 128
    n_tiles = n_slots // 128

    cores = []
    for k, pl in enumerate(percore):
        gsrc = np.zeros(n_slots, np.int32)
        gdst = np.zeros(n_slots, np.int32)
        scat = np.full(n_slots, DUMP_SLOT, np.int32)
        eid = np.full(n_slots, -1, np.int64)
        fill = dict.fromkeys(group_order, 0)
        for p in sorted(pl, key=gkey):
            g = gkey(p)
            s = slot_of[g] + fill[g]
            fill[g] += 1
            gsrc[s] = src[p["e"]]
            gdst[s] = dst[p["e"]]
            scat[s] = p["rb"] * N + p["c"]
            eid[s] = p["e"]
        cores.append({"gsrc": gsrc, "gdst": gdst, "scat": scat, "eid": eid})

    kind_t = np.full(n_slots, -1, np.int64)
    for g in group_order:
        s0 = slot_of[g]
        kind_t[s0:s0 + sizes[g]] = g[1] * 16 + g[2]
    runs = []
    for tile_i in range(n_tiles):
        s = tile_i * 128
        j = 0
        while j < 128:
            v = kind_t[s + j]
            j2 = j
            while j2 < 128 and kind_t[s + j2] == v:
                j2 += 1
            if v >= 0:
                # split into matmul-legal sub-runs. PSUM base partition must
                # be 0/32/64; a run at 96 is emitted as base-64 with 32
                # leading dummy columns (lead=32) and must be ordered before
                # the runs covering [64, 96) so those overwrite the garbage.
                a = j
                while a < j2:
                    lead = 0
                    if a == 0:
                        ln = j2 - a
                    elif a == 64:
                        ln = min(64, j2 - a)
                    elif a == 96:
                        ln = min(32, j2 - a)
                        lead = 32
                    else:
                        assert a % 32 == 0, (tile_i, a)
                        ln = min(32, j2 - a)
                    runs.append((tile_i, a, ln, int(v) // 16, int(v) % 16,
                                 lead))
                    a += ln
            j = j2
    return {"n_slots": n_slots, "n_tiles": n_tiles, "runs": runs,
            "cores": cores}


# ---------------- bass program ----------------
def _build_bass(n_tiles, runs):
    import concourse.bass as bass
    import concourse.mybir as mybir
    import concourse.tile as tile
    from concourse import bacc
    from concourse.masks import make_identity

    fp32 = mybir.dt.float32
    i32 = mybir.dt.int32
    AF = mybir.ActivationFunctionType
    OP = mybir.AluOpType
    n_slots = n_tiles * 128
    n_chunks = (n_slots + 511) // 512

    nc = bacc.Bacc()
    dp = nc.declare_dram_parameter
    nfT = dp("nfT", [F, N], fp32, isOutput=False)
    nf = dp("nf", [N, F], fp32, isOutput=False)
    naT = dp("naT", [T, N], fp32, isOutput=False)
    A_ = dp("A", [N, N], fp32, isOutput=False)
    Wmsg = dp("Wmsg", [F, F], fp32, isOutput=False)
    Wattr = dp("Wattr", [T, F], fp32, isOutput=False)
    W1 = dp("W1", [F, H], fp32, isOutput=False)
    W2 = dp("W2", [F, H], fp32, isOutput=False)
    W34 = dp("W34", [2 * EA, H], fp32, isOutput=False)
    Wp0 = dp("Wp0", [H, KDIM], fp32, isOutput=False)
    Wp1 = dp("Wp1", [F, KDIM], fp32, isOutput=False)
    Wp2 = dp("Wp2", [F, KDIM], fp32, isOutput=False)
    WeT = dp("WeT", [P * I_EDGE, KDIM], fp32, isOutput=False)
    cobF = dp("cobF", [P * I_EDGE, I_EDGE], fp32, isOutput=False)
    cobFs = dp("cobFs", [P * I_EDGE, I_EDGE], fp32, isOutput=False)
    WnT = dp("WnT", [T * I_NODE, F], fp32, isOutput=False)
    cobnF = dp("cobnF", [T * I_NODE, I_EDGE], fp32, isOutput=False)
    masks = dp("masks", [128, T], fp32, isOutput=False)
    gsrc = dp("gsrc", [128, n_tiles], i32, isOutput=False)
    gdst = dp("gdst", [128, n_tiles], i32, isOutput=False)
    scat = dp("scat", [128, n_tiles], i32, isOutput=False)
    nbgath = dp("nbgath", [128, 1], i32, isOutput=False)
    nbscat = dp("nbscat", [128, 1], i32, isOutput=False)
    efeaT = dp("efeaT", [2 * EA, n_slots], fp32, isOutput=False)
    outs = [dp(f"out{x}", [NPC + 1, N * D], fp32, isOutput=True)
            for x in range(D)]
    node_h_dram = nc.dram_tensor("node_h_dram", [N, F], fp32)

    with tile.TileContext(nc) as tc:
        with (
            tc.tile_pool(name="const", bufs=1) as const,
            tc.tile_pool(name="gbuf", bufs=1) as gbuf,
            tc.tile_pool(name="work", bufs=3) as work,
            tc.tile_pool(name="blkpool", bufs=6) as blkpool,
        ):
            # ---------- zero-fill the 13 row-family outputs ----------
            ztile = const.tile([128, 3250], fp32)
            nc.vector.memset(ztile[:], 0.0)
            for x in range(D):
                for q in range(4):
                    nc.sync.dma_start(
                        out=outs[x][0:NPC + 1, q * 3250:(q + 1) * 3250],
                        in_=ztile[0:NPC + 1, :])

            # ---------- constants in ----------
            ident = const.tile([128, 128], fp32)
            make_identity(nc, ident[:])
            def _load(name, shape, srcap):
                t = const.tile(shape, fp32, tag=name)
                nc.sync.dma_start(out=t[:], in_=srcap)
                return t
            nfT_sb = _load("nfT_sb", [F, N], nfT[:])
            naT_sb = _load("naT_sb", [T, N], naT[:])
            Wmsg_sb = _load("Wmsg_sb", [F, F], Wmsg[:])
            Wattr_sb = _load("Wattr_sb", [T, F], Wattr[:])
            W1_sb = _load("W1_sb", [F, H], W1[:])
            W2_sb = _load("W2_sb", [F, H], W2[:])
            W34_sb = _load("W34_sb", [2 * EA, H], W34[:])
            Wp0_sb = _load("Wp0_sb", [H, KDIM], Wp0[:])
            Wp1_sb = _load("Wp1_sb", [F, KDIM], Wp1[:])
            Wp2_sb = _load("Wp2_sb", [F, KDIM], Wp2[:])
            masks_sb = _load("masks_sb", [128, T], masks[:])
            def _loadi(name, shape, srcap):
                t = const.tile(shape, i32, tag=name)
                nc.sync.dma_start(out=t[:], in_=srcap)
                return t
            gsrc_sb = _loadi("gsrc_sb", [128, n_tiles], gsrc[:])
            gdst_sb = _loadi("gdst_sb", [128, n_tiles], gdst[:])
            scat_sb = _loadi("scat_sb", [128, n_tiles], scat[:])
            nbgath_sb = _loadi("nbgath_sb", [128, 1], nbgath[:])
            nbscat_sb = _loadi("nbscat_sb", [128, 1], nbscat[:])

            # ---------- phase N: node_h ----------
            with (tc.tile_pool(name="npool", bufs=2) as npool,
                  tc.tile_pool(name="apool", bufs=1) as apool,
                  tc.tile_pool(name="psN", bufs=2, space="PSUM") as psN):
                nh_sb = apool.tile([128, 8 * F], fp32)
                m_sb = apool.tile([128, 8 * F], fp32)
                NT = 125  # node tile size: 1000 = 8 * 125
                for i in range(8):
                    ps = psN.tile([NT, F], fp32, space="PSUM", tag="mps")
                    nc.tensor.matmul(out=ps[:], lhsT=nfT_sb[:, i * NT:(i + 1) * NT],
                                     rhs=Wmsg_sb[:], start=True, stop=True)
                    nc.vector.tensor_copy(out=m_sb[:NT, i * F:(i + 1) * F], in_=ps[:])
                A_sb = []
                for j in range(8):
                    at = apool.tile([NT, N], fp32, tag=f"a{j}")
                    nc.sync.dma_start(out=at[:], in_=A_[j * NT:(j + 1) * NT, :])
                    A_sb.append(at)
                for i in range(8):
                    agg = psN.tile([NT, F], fp32, space="PSUM", tag="aggps")
                    for j in range(8):
                        nc.tensor.matmul(
                            out=agg[:], lhsT=A_sb[j][:, i * NT:(i + 1) * NT],
                            rhs=m_sb[:NT, j * F:(j + 1) * F],
                            start=(j == 0), stop=(j == 7))
                    att = psN.tile([NT, F], fp32, space="PSUM", tag="attps")
                    nc.tensor.matmul(out=att[:],
                                     lhsT=naT_sb[:, i * NT:(i + 1) * NT],
                                     rhs=Wattr_sb[:], start=True, stop=True)
                    nf_t = npool.tile([NT, F], fp32, tag="nft")
                    nc.sync.dma_start(out=nf_t[:], in_=nf[i * NT:(i + 1) * NT, :])
                    nh_i = nh_sb[:NT, i * F:(i + 1) * F]
                    nc.scalar.activation(nh_i, agg[:], AF.Copy, scale=0.05)
                    nc.vector.tensor_add(out=nh_i, in0=nh_i, in1=nf_t[:])
                    nc.vector.tensor_add(out=nh_i, in0=nh_i, in1=att[:])
                    nc.sync.dma_start(out=node_h_dram[i * NT:(i + 1) * NT, :],
                                      in_=nh_i)

            # ---------- phase C: C1/C2/Ccomb and D ----------
            C1_sb = const.tile([KDIM, P * I_EDGE], fp32)
            C2_sb = const.tile([KDIM, P * I_EDGE], fp32)
            Ccb_sb = const.tile([KDIM, P * I_EDGE], fp32)
            D_sb = const.tile([F, T * I_EDGE], fp32)
            with (tc.tile_pool(name="cpool", bufs=3) as cpool,
                  tc.tile_pool(name="psC2", bufs=2, space="PSUM") as psC2):
                for t in range(P):
                    wa = cpool.tile([128, KDIM], fp32, tag="wa")
                    nc.sync.dma_start(out=wa[:], in_=WeT[t * 169:t * 169 + 128, :])
                    wb = cpool.tile([41, KDIM], fp32, tag="wb")
                    nc.sync.dma_start(out=wb[:], in_=WeT[t * 169 + 128:(t + 1) * 169, :])
                    for ci, csrc in enumerate((cobF, cobFs)):
                        ca = cpool.tile([128, I_EDGE], fp32, tag="ca")
                        nc.sync.dma_start(out=ca[:], in_=csrc[t * 169:t * 169 + 128, :])
                        cb = cpool.tile([41, I_EDGE], fp32, tag="cb")
                        nc.sync.dma_start(out=cb[:],
                                          in_=csrc[t * 169 + 128:(t + 1) * 169, :])
                        ps = psC2.tile([KDIM, I_EDGE], fp32, space="PSUM", tag="cps")
                        nc.tensor.matmul(out=ps[:], lhsT=wa[:], rhs=ca[:],
                                         start=True, stop=False)
                        nc.tensor.matmul(out=ps[:], lhsT=wb[:], rhs=cb[:],
                                         start=False, stop=True)
                        dst = (C1_sb if ci == 0 else C2_sb)
                        nc.scalar.activation(dst[:, t * 169:(t + 1) * 169], ps[:],
                                             AF.Copy, scale=0.5)
                for tau in range(T):
                    wn = cpool.tile([I_NODE, F], fp32, tag="wn")
                    nc.sync.dma_start(out=wn[:],
                                      in_=WnT[tau * I_NODE:(tau + 1) * I_NODE, :])
                    cn = cpool.tile([I_NODE, I_EDGE], fp32, tag="cn")
                    nc.sync.dma_start(out=cn[:],
                                      in_=cobnF[tau * I_NODE:(tau + 1) * I_NODE, :])
                    ps = psC2.tile([F, I_EDGE], fp32, space="PSUM", tag="dps")
                    nc.tensor.matmul(out=ps[:], lhsT=wn[:], rhs=cn[:],
                                     start=True, stop=True)
                    nc.vector.tensor_copy(out=D_sb[:, tau * 169:(tau + 1) * 169],
                                          in_=ps[:])
            nc.vector.tensor_add(out=Ccb_sb[:], in0=C1_sb[:], in1=C2_sb[:])
            nc.vector.tensor_scalar_mul(Ccb_sb[:], Ccb_sb[:], 2.0)

            # ---------- phase NB: node diagonal blocks ----------
            with tc.tile_pool(name="psNB", bufs=2, space="PSUM") as psNB:
                nbg = gbuf.tile([128, F], fp32)
                nc.gpsimd.indirect_dma_start(
                    out=nbg[:], out_offset=None, in_=node_h_dram[:],
                    in_offset=bass.IndirectOffsetOnAxis(ap=nbgath_sb[:, :1], axis=0))
                nbt_ps = psNB.tile([F, 128], fp32, space="PSUM", tag="nbt")
                nc.tensor.transpose(out=nbt_ps[:], in_=nbg[:], identity=ident[:])
                nhT_own = gbuf.tile([F, 128], fp32)
                nc.vector.tensor_copy(out=nhT_own[:], in_=nbt_ps[:])
                NB_sb = gbuf.tile([128, I_EDGE], fp32)
                tmp_nb = gbuf.tile([128, I_EDGE], fp32)
                for tau in range(T):
                    ps = psNB.tile([128, I_EDGE], fp32, space="PSUM", tag="nbps")
                    nc.tensor.matmul(out=ps[:], lhsT=nhT_own[:],
                                     rhs=D_sb[:, tau * 169:(tau + 1) * 169],
                                     start=True, stop=True)
                    if tau == 0:
                        nc.vector.tensor_tensor(
                            out=NB_sb[:], in0=ps[:],
                            in1=masks_sb[:, 0:1].to_broadcast([128, I_EDGE]),
                            op=OP.mult)
                    else:
                        nc.vector.tensor_tensor(
                            out=tmp_nb[:], in0=ps[:],
                            in1=masks_sb[:, tau:tau + 1].to_broadcast([128, I_EDGE]),
                            op=OP.mult)
                        nc.vector.tensor_add(out=NB_sb[:], in0=NB_sb[:],
                                             in1=tmp_nb[:])

            # ---------- NB scatter ----------
            for x in range(D):
                ofl = outs[x][:].rearrange("r (cb y) -> (r cb) y", y=D)
                nc.gpsimd.indirect_dma_start(
                    out=ofl,
                    out_offset=bass.IndirectOffsetOnAxis(ap=nbscat_sb[:, :1],
                                                         axis=0),
                    in_=NB_sb[:, x * D:(x + 1) * D], in_offset=None,
                    compute_op=OP.add)

            # ---------- phase E: gathers + em + h (chunk-fused) ----------
            hij_sb = gbuf.tile([KDIM, n_slots], fp32)
            hji_sb = gbuf.tile([KDIM, n_slots], fp32)
            with tc.tile_pool(name="psE", bufs=2, space="PSUM") as psE:
                for c in range(n_chunks):
                    c0 = c * 512
                    cw = min(512, n_slots - c0)
                    cs = slice(c0, c0 + cw)
                    gst = work.tile([F, 512], fp32, tag="gstc")
                    gdt = work.tile([F, 512], fp32, tag="gdtc")
                    for ti in range(c0 // 128, (c0 + cw) // 128):
                        o = ti * 128 - c0
                        for idx_sb, dstT in ((gsrc_sb, gst), (gdst_sb, gdt)):
                            g = work.tile([128, F], fp32, tag="gg")
                            nc.gpsimd.indirect_dma_start(
                                out=g[:], out_offset=None, in_=node_h_dram[:],
                                in_offset=bass.IndirectOffsetOnAxis(
                                    ap=idx_sb[:, ti:ti + 1], axis=0))
                            tps = psE.tile([F, 128], fp32, space="PSUM", tag="tps")
                            nc.tensor.transpose(out=tps[:], in_=g[:],
                                                identity=ident[:])
                            nc.vector.tensor_copy(
                                out=dstT[:, o:o + 128], in_=tps[:])
                    efc = work.tile([2 * EA, 512], fp32, tag="efc")
                    nc.sync.dma_start(out=efc[:, :cw], in_=efeaT[:, cs])
                    em_ps = psE.tile([H, 512], fp32, space="PSUM", tag="emps")
                    nc.tensor.matmul(out=em_ps[:, :cw], lhsT=W1_sb[:],
                                     rhs=gst[:, :cw], start=True, stop=False)
                    nc.tensor.matmul(out=em_ps[:, :cw], lhsT=W2_sb[:],
                                     rhs=gdt[:, :cw], start=False, stop=False)
                    nc.tensor.matmul(out=em_ps[:, :cw], lhsT=W34_sb[:],
                                     rhs=efc[:, :cw], start=False, stop=True)
                    emT = work.tile([H, 512], fp32, tag="emT")
                    nc.scalar.activation(emT[:, :cw], em_ps[:, :cw], AF.Tanh)
                    hij_ps = psE.tile([KDIM, 512], fp32, space="PSUM", tag="hijps")
                    nc.tensor.matmul(out=hij_ps[:, :cw], lhsT=Wp0_sb[:],
                                     rhs=emT[:, :cw], start=True, stop=False)
                    nc.tensor.matmul(out=hij_ps[:, :cw], lhsT=Wp1_sb[:],
                                     rhs=gst[:, :cw], start=False, stop=False)
                    nc.tensor.matmul(out=hij_ps[:, :cw], lhsT=Wp2_sb[:],
                                     rhs=gdt[:, :cw], start=False, stop=True)
                    nc.vector.tensor_copy(out=hij_sb[:, cs], in_=hij_ps[:, :cw])
                    hji_ps = psE.tile([KDIM, 512], fp32, space="PSUM", tag="hjips")
                    nc.tensor.matmul(out=hji_ps[:, :cw], lhsT=Wp0_sb[:],
                                     rhs=emT[:, :cw], start=True, stop=False)
                    nc.tensor.matmul(out=hji_ps[:, :cw], lhsT=Wp1_sb[:],
                                     rhs=gdt[:, :cw], start=False, stop=False)
                    nc.tensor.matmul(out=hji_ps[:, :cw], lhsT=Wp2_sb[:],
                                     rhs=gst[:, :cw], start=False, stop=True)
                    nc.vector.tensor_copy(out=hji_sb[:, cs], in_=hji_ps[:, :cw])

            # ---------- blocks + scatter ----------
            runs_by_tile = {}
            for (ti, col0, ln, kind, t, lead) in runs:
                runs_by_tile.setdefault(ti, []).append((col0, ln, kind, t, lead))
            for ti in runs_by_tile:
                # stomp-runs (lead>0) first so later runs overwrite their lead
                runs_by_tile[ti].sort(key=lambda r: -r[4])
            with tc.tile_pool(name="psBK", bufs=2, space="PSUM") as psBK:
                for ti in range(n_tiles):
                    bps = psBK.tile([128, I_EDGE], fp32, space="PSUM", tag="bps")
                    nc.vector.tensor_copy(out=bps[:], in_=ztile[:, :I_EDGE])
                    for (col0, ln, kind, t, lead) in runs_by_tile.get(ti, []):
                        sl = slice(ti * 128 + col0 - lead,
                                   ti * 128 + col0 + ln)
                        oap = bps[col0 - lead:col0 + ln, :]
                        tt = slice(t * 169, (t + 1) * 169)
                        if kind == 0:
                            nc.tensor.matmul(out=oap, lhsT=hij_sb[:, sl],
                                             rhs=C1_sb[:, tt], start=True,
                                             stop=False)
                            nc.tensor.matmul(out=oap, lhsT=hji_sb[:, sl],
                                             rhs=C2_sb[:, tt], start=False,
                                             stop=True)
                        elif kind == 1:
                            nc.tensor.matmul(out=oap, lhsT=hji_sb[:, sl],
                                             rhs=C1_sb[:, tt], start=True,
                                             stop=False)
                            nc.tensor.matmul(out=oap, lhsT=hij_sb[:, sl],
                                             rhs=C2_sb[:, tt], start=False,
                                             stop=True)
                        else:
                            nc.tensor.matmul(out=oap, lhsT=hij_sb[:, sl],
                                             rhs=Ccb_sb[:, tt], start=True,
                                             stop=True)
                    blk = blkpool.tile([128, I_EDGE], fp32, tag="blk")
                    nc.vector.tensor_copy(out=blk[:], in_=bps[:])
                    for x in range(D):
                        ofl = outs[x][:].rearrange("r (cb y) -> (r cb) y", y=D)
                        nc.gpsimd.indirect_dma_start(
                            out=ofl,
                            out_offset=bass.IndirectOffsetOnAxis(
                                ap=scat_sb[:, ti:ti + 1], axis=0),
                            in_=blk[:, x * D:(x + 1) * D], in_offset=None,
                            compute_op=OP.add)
    nc.finalize()
    return nc


# ---------------- host inputs per core ----------------
def _make_in_maps(inputs, sched):
    src = inputs["edge_index"][0].astype(np.int64)
    dst = inputs["edge_index"][1].astype(np.int64)
    ntypes = inputs["node_types"].astype(np.int64)
    A = _build_adjacency(src, dst)
    n_slots = sched["n_slots"]
    n_tiles = sched["n_tiles"]

    nf = np.ascontiguousarray(inputs["node_feats"], np.float32)
    shared = {
        "nfT": np.ascontiguousarray(nf.T),
        "nf": nf,
        "naT": np.ascontiguousarray(inputs["node_attrs"].T, dtype=np.float32),
        "A": A,
        "Wmsg": np.ascontiguousarray(inputs["W_msg"], np.float32),
        "Wattr": np.ascontiguousarray(inputs["W_attr"], np.float32),
        "W1": np.ascontiguousarray(inputs["W_em"][:F], np.float32),
        "W2": np.ascontiguousarray(inputs["W_em"][F:2 * F], np.float32),
        "W34": np.ascontiguousarray(inputs["W_em"][2 * F:], np.float32),
        "Wp0": np.ascontiguousarray(inputs["W_proj"][:H], np.float32),
        "Wp1": np.ascontiguousarray(inputs["W_proj"][H:H + F], np.float32),
        "Wp2": np.ascontiguousarray(inputs["W_proj"][H + F:], np.float32),
        "WeT": np.ascontiguousarray(
            inputs["W_edge"].transpose(0, 2, 1).reshape(P * I_EDGE, KDIM)),
        "cobF": np.ascontiguousarray(
            inputs["cob_edge"].reshape(P, I_EDGE, I_EDGE)
            .reshape(P * I_EDGE, I_EDGE)),
        "cobFs": np.ascontiguousarray(
            inputs["cob_edge"].transpose(0, 1, 3, 2)
            .reshape(P * I_EDGE, I_EDGE)),
        "WnT": np.ascontiguousarray(
            inputs["W_node"].transpose(0, 2, 1).reshape(T * I_NODE, F)),
        "cobnF": np.ascontiguousarray(
            inputs["cob_node"].reshape(T, I_NODE, I_EDGE)
            .reshape(T * I_NODE, I_EDGE)),
    }
    ef = np.concatenate([inputs["edge_feats"], inputs["edge_attrs"]],
                        axis=1).astype(np.float32)  # [E, 32]

    in_maps = []
    for k in range(NCORES):
        c = sched["cores"][k]
        base = k * NPC
        eid = c["eid"]
        efea = np.zeros((n_slots, 2 * EA), np.float32)
        valid = eid >= 0
        efea[valid] = ef[eid[valid]]
        masks = np.zeros((128, T), np.float32)
        own_t = ntypes[base:base + NPC]
        for tau in range(T):
            masks[:NPC, tau] = (own_t == tau)
        nbgath = np.zeros((128, 1), np.int32)
        nbgath[:NPC, 0] = np.arange(base, base + NPC)
        nbscat = np.full((128, 1), DUMP_SLOT, np.int32)
        nbscat[:NPC, 0] = np.arange(NPC) * N + np.arange(base, base + NPC)
        im = dict(shared)
        im.update({
            "masks": masks,
            "gsrc": np.ascontiguousarray(
                c["gsrc"].reshape(n_tiles, 128).T.astype(np.int32)),
            "gdst": np.ascontiguousarray(
                c["gdst"].reshape(n_tiles, 128).T.astype(np.int32)),
            "scat": np.ascontiguousarray(
                c["scat"].reshape(n_tiles, 128).T.astype(np.int32)),
            "nbgath": nbgath,
            "nbscat": nbscat,
            "efeaT": np.ascontiguousarray(efea.T),
        })
        in_maps.append(im)
    return in_maps


_CACHE = {}


def kernel(**inputs):
    from concourse.bass_utils import run_bass_kernel_spmd

    inputs = {k: np.asarray(v) for k, v in inputs.items()}
    sched = _build_schedule(inputs["edge_index"], inputs["edge_types"])
    key = (sched["n_tiles"], tuple(sched["runs"]))
    if key not in _CACHE:
        _CACHE[key] = _build_bass(sched["n_tiles"], sched["runs"])
    nc = _CACHE[key]
    in_maps = _make_in_maps(inputs, sched)
    res = run_bass_kernel_spmd(nc, in_maps, list(range(NCORES)))
    shards = []
    for k in range(NCORES):
        fam = np.stack([res.results[k][f"out{x}"][:NPC] for x in range(D)],
                       axis=1)  # [125, 13, 13000]
        shards.append(fam.reshape(NPC * D, N * D))
    return np.concatenate(shards, axis=0)


if __name__ == "__main__":
    import jax
    sys.path.insert(0, "/root/problem")
    import reference
    with jax.default_device(jax.local_devices(backend="cpu")[0]):
        inp = {k: np.asarray(v) for k, v in reference.setup_inputs().items()}
        expected = np.asarray(reference.reference(**inp))
    got = kernel(**inp)
    err = np.abs(got - expected).max()
    rel = err / np.abs(expected).max()
    print("max abs err", err, "rel", rel)
    print("PASS" if rel < 1e-4 else "FAIL")


# revision 7
# speedup vs baseline: 2.2499x; 2.2499x over previous
"""Trainium2 Bass kernel for nn_BasisMatrixReadout (GNN message passing ->
dense symmetric block matrix readout).

Strategy (8 NeuronCores, SPMD):
  - Output M [13000, 13000] f32 sharded by node row-blocks: core k owns nodes
    [125k, 125k+125) -> rows [1625k, 1625k+1625).
  - Rows are further split into 13 row-family tensors out_x (row r = 13*rb+x)
    so scatter-call chains to different families are independent.
  - All float math on device: node_h via adjacency-count matmul (segment_sum
    == A @ m), edge messages / projections as feature-major matmuls, per-type
    block ops folded into C1 = 0.5*W_edge@cob, C2 = 0.5*W_edge@cob^T(xy),
    Ccomb = 2*(C1+C2) for self-edges; node diagonal blocks via
    D_tau = W_node@cob_node.
  - Host does integer-only schedule construction (placement lists sorted by
    (collision-wave, kind, edge-type), padded to cross-core maxima so a single
    SPMD program serves all cores); gathers use indirect DMA with index
    tensors; block scatter uses indirect DMA (13-float rows) with CCE add onto
    zero-filled outputs; collision waves live in dedicated trailing tiles so
    no two descriptors of one call target the same address.
"""
import sys

sys.path.insert(0, "/opt/trn_rl_repo")
import numpy as np

# ---------------- problem constants (hardcoded per spec) ----------------
N, E, T, P, D = 1000, 20000, 4, 10, 13
F, H, EA, KDIM = 128, 128, 16, 64
I_EDGE = D * D            # 169
I_NODE = D * (D + 1) // 2  # 91
NCORES = 8
NPC = N // NCORES         # 125 nodes per core
DUMP_SLOT = NPC * N       # dump row (row 125) slot base in out_x
KINDS = ("fwd", "rev", "self")


# ---------------- host-side integer schedule ----------------
def _build_adjacency(src, dst):
    A = np.zeros((N, N), np.float32)
    np.add.at(A, (dst, src), 1.0)
    np.add.at(A, (src, dst), 1.0)
    return A


def _core_placements(src, dst, etype, k):
    base = k * NPC
    hi = base + NPC
    pl = []
    for kind, mask in (("fwd", (src >= base) & (src < hi) & (src != dst)),
                       ("rev", (dst >= base) & (dst < hi) & (src != dst)),
                       ("self", (src == dst) & (src >= base) & (src < hi))):
        for e in np.nonzero(mask)[0]:
            if kind == "fwd":
                rb, c = src[e] - base, dst[e]
            elif kind == "rev":
                rb, c = dst[e] - base, src[e]
            else:
                rb, c = src[e] - base, src[e]
            pl.append({"kind": kind, "t": int(etype[e]), "rb": int(rb),
                       "c": int(c), "e": int(e)})
    counts = {}
    for p in pl:
        tgt = (p["rb"], p["c"])
        w = counts.get(tgt, 1 if p["c"] == base + p["rb"] else 0)
        p["wave"] = w
        counts[tgt] = w + 1
    return pl


def _build_schedule(edge_index, edge_types):
    src = edge_index[0].astype(np.int64)
    dst = edge_index[1].astype(np.int64)
    et = edge_types.astype(np.int64)
    percore = [_core_placements(src, dst, et, k) for k in range(NCORES)]

    def gkey(p):
        return (p["wave"], KINDS.index(p["kind"]), p["t"])

    sizes = {}
    for pl in percore:
        cnt = {}
        for p in pl:
            cnt[gkey(p)] = cnt.get(gkey(p), 0) + 1
        for kk, v in cnt.items():
            sizes[kk] = max(sizes.get(kk, 0), v)
    # pad groups to multiples of 32 so matmul sub-runs start 32-aligned
    for kk in sizes:
        sizes[kk] = (sizes[kk] + 31) // 32 * 32

    group_order = sorted(sizes)
    slot_of = {}
    cursor = 0
    cur_wave = 0
    for g in group_order:
        if g[0] != cur_wave:
            cursor = (cursor + 127) // 128 * 128  # wave -> tile boundary
            cur_wave = g[0]
        slot_of[g] = cursor
        cursor += sizes[g]
    n_slots = (cursor + 127) // 128 * 128
    n_tiles = n_slots // 128

    cores = []
    for k, pl in enumerate(percore):
        gsrc = np.zeros(n_slots, np.int32)
        gdst = np.zeros(n_slots, np.int32)
        scat = np.full(n_slots, DUMP_SLOT, np.int32)
        eid = np.full(n_slots, -1, np.int64)
        fill = dict.fromkeys(group_order, 0)
        for p in sorted(pl, key=gkey):
            g = gkey(p)
            s = slot_of[g] + fill[g]
            fill[g] += 1
            gsrc[s] = src[p["e"]]
            gdst[s] = dst[p["e"]]
            scat[s] = p["rb"] * N + p["c"]
            eid[s] = p["e"]
        cores.append({"gsrc": gsrc, "gdst": gdst, "scat": scat, "eid": eid})

    kind_t = np.full(n_slots, -1, np.int64)
    for g in group_order:
        s0 = slot_of[g]
        kind_t[s0:s0 + sizes[g]] = g[1] * 16 + g[2]
    runs = []
    for tile_i in range(n_tiles):
        s = tile_i * 128
        j = 0
        while j < 128:
            v = kind_t[s + j]
            j2 = j
            while j2 < 128 and kind_t[s + j2] == v:
                j2 += 1
            if v >= 0:
                # split into matmul-legal sub-runs. PSUM base partition must
                # be 0/32/64; a run at 96 is emitted as base-64 with 32
                # leading dummy columns (lead=32) and must be ordered before
                # the runs covering [64, 96) so those overwrite the garbage.
                a = j
                while a < j2:
                    lead = 0
                    if a == 0:
                        ln = j2 - a
                    elif a == 64:
                        ln = min(64, j2 - a)
                    elif a == 96:
                        ln = min(32, j2 - a)
                        lead = 32
                    else:
                        assert a % 32 == 0, (tile_i, a)
                        ln = min(32, j2 - a)
                    runs.append((tile_i, a, ln, int(v) // 16, int(v) % 16,
                                 lead))
                    a += ln
            j = j2
    return {"n_slots": n_slots, "n_tiles": n_tiles, "runs": runs,
            "cores": cores}


# ---------------- bass program ----------------
def _build_bass(n_tiles, runs):
    import concourse.bass as bass
    import concourse.mybir as mybir
    import concourse.tile as tile
    from concourse import bacc
    from concourse.masks import make_identity

    fp32 = mybir.dt.float32
    i32 = mybir.dt.int32
    AF = mybir.ActivationFunctionType
    OP = mybir.AluOpType
    n_slots = n_tiles * 128
    n_chunks = (n_slots + 511) // 512

    nc = bacc.Bacc()
    dp = nc.declare_dram_parameter
    nfT = dp("nfT", [F, N], fp32, isOutput=False)
    nf = dp("nf", [N, F], fp32, isOutput=False)
    naT = dp("naT", [T, N], fp32, isOutput=False)
    A_ = dp("A", [N, N], fp32, isOutput=False)
    Wmsg = dp("Wmsg", [F, F], fp32, isOutput=False)
    Wattr = dp("Wattr", [T, F], fp32, isOutput=False)
    W1 = dp("W1", [F, H], fp32, isOutput=False)
    W2 = dp("W2", [F, H], fp32, isOutput=False)
    W34 = dp("W34", [2 * EA, H], fp32, isOutput=False)
    Wp0 = dp("Wp0", [H, KDIM], fp32, isOutput=False)
    Wp1 = dp("Wp1", [F, KDIM], fp32, isOutput=False)
    Wp2 = dp("Wp2", [F, KDIM], fp32, isOutput=False)
    WeT = dp("WeT", [P * I_EDGE, KDIM], fp32, isOutput=False)
    cobF = dp("cobF", [P * I_EDGE, I_EDGE], fp32, isOutput=False)
    cobFs = dp("cobFs", [P * I_EDGE, I_EDGE], fp32, isOutput=False)
    WnT = dp("WnT", [T * I_NODE, F], fp32, isOutput=False)
    cobnF = dp("cobnF", [T * I_NODE, I_EDGE], fp32, isOutput=False)
    masks = dp("masks", [128, T], fp32, isOutput=False)
    gsrc = dp("gsrc", [128, n_tiles], i32, isOutput=False)
    gdst = dp("gdst", [128, n_tiles], i32, isOutput=False)
    scat = dp("scat", [128, n_tiles], i32, isOutput=False)
    nbgath = dp("nbgath", [128, 1], i32, isOutput=False)
    nbscat = dp("nbscat", [128, 1], i32, isOutput=False)
    efeaT = dp("efeaT", [2 * EA, n_slots], fp32, isOutput=False)
    fam = dp("fam", [NPC + 1, N * I_EDGE], fp32, isOutput=True)
    node_h_dram = nc.dram_tensor("node_h_dram", [N, F], fp32)

    with tile.TileContext(nc) as tc:
        with (
            tc.tile_pool(name="const", bufs=1) as const,
            tc.tile_pool(name="gbuf", bufs=1) as gbuf,
            tc.tile_pool(name="work", bufs=3) as work,
            tc.tile_pool(name="blkpool", bufs=6) as blkpool,
        ):
            # ---------- zero-fill the 13 row-family outputs ----------
            ztile = const.tile([128, 3380], fp32)
            nc.vector.memset(ztile[:], 0.0)
            for q in range(50):  # 50 * 3380 = 169000
                nc.sync.dma_start(
                    out=fam[0:NPC + 1, q * 3380:(q + 1) * 3380],
                    in_=ztile[0:NPC + 1, :])

            # ---------- constants in ----------
            ident = const.tile([128, 128], fp32)
            make_identity(nc, ident[:])
            def _load(name, shape, srcap):
                t = const.tile(shape, fp32, tag=name)
                nc.sync.dma_start(out=t[:], in_=srcap)
                return t
            nfT_sb = _load("nfT_sb", [F, N], nfT[:])
            naT_sb = _load("naT_sb", [T, N], naT[:])
            Wmsg_sb = _load("Wmsg_sb", [F, F], Wmsg[:])
            Wattr_sb = _load("Wattr_sb", [T, F], Wattr[:])
            W1_sb = _load("W1_sb", [F, H], W1[:])
            W2_sb = _load("W2_sb", [F, H], W2[:])
            W34_sb = _load("W34_sb", [2 * EA, H], W34[:])
            Wp0_sb = _load("Wp0_sb", [H, KDIM], Wp0[:])
            Wp1_sb = _load("Wp1_sb", [F, KDIM], Wp1[:])
            Wp2_sb = _load("Wp2_sb", [F, KDIM], Wp2[:])
            masks_sb = _load("masks_sb", [128, T], masks[:])
            def _loadi(name, shape, srcap):
                t = const.tile(shape, i32, tag=name)
                nc.sync.dma_start(out=t[:], in_=srcap)
                return t
            gsrc_sb = _loadi("gsrc_sb", [128, n_tiles], gsrc[:])
            gdst_sb = _loadi("gdst_sb", [128, n_tiles], gdst[:])
            scat_sb = _loadi("scat_sb", [128, n_tiles], scat[:])
            nbgath_sb = _loadi("nbgath_sb", [128, 1], nbgath[:])
            nbscat_sb = _loadi("nbscat_sb", [128, 1], nbscat[:])

            # ---------- phase N: node_h ----------
            with (tc.tile_pool(name="npool", bufs=2) as npool,
                  tc.tile_pool(name="apool", bufs=1) as apool,
                  tc.tile_pool(name="psN", bufs=2, space="PSUM") as psN):
                nh_sb = apool.tile([128, 8 * F], fp32)
                m_sb = apool.tile([128, 8 * F], fp32)
                NT = 125  # node tile size: 1000 = 8 * 125
                for i in range(8):
                    ps = psN.tile([NT, F], fp32, space="PSUM", tag="mps")
                    nc.tensor.matmul(out=ps[:], lhsT=nfT_sb[:, i * NT:(i + 1) * NT],
                                     rhs=Wmsg_sb[:], start=True, stop=True)
                    nc.vector.tensor_copy(out=m_sb[:NT, i * F:(i + 1) * F], in_=ps[:])
                A_sb = []
                for j in range(8):
                    at = apool.tile([NT, N], fp32, tag=f"a{j}")
                    nc.sync.dma_start(out=at[:], in_=A_[j * NT:(j + 1) * NT, :])
                    A_sb.append(at)
                for i in range(8):
                    agg = psN.tile([NT, F], fp32, space="PSUM", tag="aggps")
                    for j in range(8):
                        nc.tensor.matmul(
                            out=agg[:], lhsT=A_sb[j][:, i * NT:(i + 1) * NT],
                            rhs=m_sb[:NT, j * F:(j + 1) * F],
                            start=(j == 0), stop=(j == 7))
                    att = psN.tile([NT, F], fp32, space="PSUM", tag="attps")
                    nc.tensor.matmul(out=att[:],
                                     lhsT=naT_sb[:, i * NT:(i + 1) * NT],
                                     rhs=Wattr_sb[:], start=True, stop=True)
                    nf_t = npool.tile([NT, F], fp32, tag="nft")
                    nc.sync.dma_start(out=nf_t[:], in_=nf[i * NT:(i + 1) * NT, :])
                    nh_i = nh_sb[:NT, i * F:(i + 1) * F]
                    nc.scalar.activation(nh_i, agg[:], AF.Copy, scale=0.05)
                    nc.vector.tensor_add(out=nh_i, in0=nh_i, in1=nf_t[:])
                    nc.vector.tensor_add(out=nh_i, in0=nh_i, in1=att[:])
                    nc.sync.dma_start(out=node_h_dram[i * NT:(i + 1) * NT, :],
                                      in_=nh_i)

            # ---------- phase C: C1/C2/Ccomb and D ----------
            C1_sb = const.tile([KDIM, P * I_EDGE], fp32)
            C2_sb = const.tile([KDIM, P * I_EDGE], fp32)
            Ccb_sb = const.tile([KDIM, P * I_EDGE], fp32)
            D_sb = const.tile([F, T * I_EDGE], fp32)
            with (tc.tile_pool(name="cpool", bufs=3) as cpool,
                  tc.tile_pool(name="psC2", bufs=2, space="PSUM") as psC2):
                for t in range(P):
                    wa = cpool.tile([128, KDIM], fp32, tag="wa")
                    nc.sync.dma_start(out=wa[:], in_=WeT[t * 169:t * 169 + 128, :])
                    wb = cpool.tile([41, KDIM], fp32, tag="wb")
                    nc.sync.dma_start(out=wb[:], in_=WeT[t * 169 + 128:(t + 1) * 169, :])
                    for ci, csrc in enumerate((cobF, cobFs)):
                        ca = cpool.tile([128, I_EDGE], fp32, tag="ca")
                        nc.sync.dma_start(out=ca[:], in_=csrc[t * 169:t * 169 + 128, :])
                        cb = cpool.tile([41, I_EDGE], fp32, tag="cb")
                        nc.sync.dma_start(out=cb[:],
                                          in_=csrc[t * 169 + 128:(t + 1) * 169, :])
                        ps = psC2.tile([KDIM, I_EDGE], fp32, space="PSUM", tag="cps")
                        nc.tensor.matmul(out=ps[:], lhsT=wa[:], rhs=ca[:],
                                         start=True, stop=False)
                        nc.tensor.matmul(out=ps[:], lhsT=wb[:], rhs=cb[:],
                                         start=False, stop=True)
                        dst = (C1_sb if ci == 0 else C2_sb)
                        nc.scalar.activation(dst[:, t * 169:(t + 1) * 169], ps[:],
                                             AF.Copy, scale=0.5)
                for tau in range(T):
                    wn = cpool.tile([I_NODE, F], fp32, tag="wn")
                    nc.sync.dma_start(out=wn[:],
                                      in_=WnT[tau * I_NODE:(tau + 1) * I_NODE, :])
                    cn = cpool.tile([I_NODE, I_EDGE], fp32, tag="cn")
                    nc.sync.dma_start(out=cn[:],
                                      in_=cobnF[tau * I_NODE:(tau + 1) * I_NODE, :])
                    ps = psC2.tile([F, I_EDGE], fp32, space="PSUM", tag="dps")
                    nc.tensor.matmul(out=ps[:], lhsT=wn[:], rhs=cn[:],
                                     start=True, stop=True)
                    nc.vector.tensor_copy(out=D_sb[:, tau * 169:(tau + 1) * 169],
                                          in_=ps[:])
            nc.vector.tensor_add(out=Ccb_sb[:], in0=C1_sb[:], in1=C2_sb[:])
            nc.vector.tensor_scalar_mul(Ccb_sb[:], Ccb_sb[:], 2.0)

            # ---------- phase NB: node diagonal blocks ----------
            with tc.tile_pool(name="psNB", bufs=2, space="PSUM") as psNB:
                nbg = gbuf.tile([128, F], fp32)
                nc.gpsimd.indirect_dma_start(
                    out=nbg[:], out_offset=None, in_=node_h_dram[:],
                    in_offset=bass.IndirectOffsetOnAxis(ap=nbgath_sb[:, :1], axis=0))
                nbt_ps = psNB.tile([F, 128], fp32, space="PSUM", tag="nbt")
                nc.tensor.transpose(out=nbt_ps[:], in_=nbg[:], identity=ident[:])
                nhT_own = gbuf.tile([F, 128], fp32)
                nc.vector.tensor_copy(out=nhT_own[:], in_=nbt_ps[:])
                NB_sb = gbuf.tile([128, I_EDGE], fp32)
                tmp_nb = gbuf.tile([128, I_EDGE], fp32)
                for tau in range(T):
                    ps = psNB.tile([128, I_EDGE], fp32, space="PSUM", tag="nbps")
                    nc.tensor.matmul(out=ps[:], lhsT=nhT_own[:],
                                     rhs=D_sb[:, tau * 169:(tau + 1) * 169],
                                     start=True, stop=True)
                    if tau == 0:
                        nc.vector.tensor_tensor(
                            out=NB_sb[:], in0=ps[:],
                            in1=masks_sb[:, 0:1].to_broadcast([128, I_EDGE]),
                            op=OP.mult)
                    else:
                        nc.vector.tensor_tensor(
                            out=tmp_nb[:], in0=ps[:],
                            in1=masks_sb[:, tau:tau + 1].to_broadcast([128, I_EDGE]),
                            op=OP.mult)
                        nc.vector.tensor_add(out=NB_sb[:], in0=NB_sb[:],
                                             in1=tmp_nb[:])

            # ---------- NB scatter (one whole-block call) ----------
            fam_fl = fam[:].rearrange("r (cb e) -> (r cb) e", e=I_EDGE)
            nc.gpsimd.indirect_dma_start(
                out=fam_fl,
                out_offset=bass.IndirectOffsetOnAxis(ap=nbscat_sb[:, :1],
                                                     axis=0),
                in_=NB_sb[:], in_offset=None, compute_op=OP.add)

            # ---------- phase E: gathers + em + h (chunk-fused) ----------
            hij_sb = gbuf.tile([KDIM, n_slots], fp32)
            hji_sb = gbuf.tile([KDIM, n_slots], fp32)
            with tc.tile_pool(name="psE", bufs=2, space="PSUM") as psE:
                for c in range(n_chunks):
                    c0 = c * 512
                    cw = min(512, n_slots - c0)
                    cs = slice(c0, c0 + cw)
                    gst = work.tile([F, 512], fp32, tag="gstc")
                    gdt = work.tile([F, 512], fp32, tag="gdtc")
                    for ti in range(c0 // 128, (c0 + cw) // 128):
                        o = ti * 128 - c0
                        for idx_sb, dstT in ((gsrc_sb, gst), (gdst_sb, gdt)):
                            g = work.tile([128, F], fp32, tag="gg")
                            nc.gpsimd.indirect_dma_start(
                                out=g[:], out_offset=None, in_=node_h_dram[:],
                                in_offset=bass.IndirectOffsetOnAxis(
                                    ap=idx_sb[:, ti:ti + 1], axis=0))
                            tps = psE.tile([F, 128], fp32, space="PSUM", tag="tps")
                            nc.tensor.transpose(out=tps[:], in_=g[:],
                                                identity=ident[:])
                            nc.vector.tensor_copy(
                                out=dstT[:, o:o + 128], in_=tps[:])
                    efc = work.tile([2 * EA, 512], fp32, tag="efc")
                    nc.sync.dma_start(out=efc[:, :cw], in_=efeaT[:, cs])
                    em_ps = psE.tile([H, 512], fp32, space="PSUM", tag="emps")
                    nc.tensor.matmul(out=em_ps[:, :cw], lhsT=W1_sb[:],
                                     rhs=gst[:, :cw], start=True, stop=False)
                    nc.tensor.matmul(out=em_ps[:, :cw], lhsT=W2_sb[:],
                                     rhs=gdt[:, :cw], start=False, stop=False)
                    nc.tensor.matmul(out=em_ps[:, :cw], lhsT=W34_sb[:],
                                     rhs=efc[:, :cw], start=False, stop=True)
                    emT = work.tile([H, 512], fp32, tag="emT")
                    nc.scalar.activation(emT[:, :cw], em_ps[:, :cw], AF.Tanh)
                    hij_ps = psE.tile([KDIM, 512], fp32, space="PSUM", tag="hijps")
                    nc.tensor.matmul(out=hij_ps[:, :cw], lhsT=Wp0_sb[:],
                                     rhs=emT[:, :cw], start=True, stop=False)
                    nc.tensor.matmul(out=hij_ps[:, :cw], lhsT=Wp1_sb[:],
                                     rhs=gst[:, :cw], start=False, stop=False)
                    nc.tensor.matmul(out=hij_ps[:, :cw], lhsT=Wp2_sb[:],
                                     rhs=gdt[:, :cw], start=False, stop=True)
                    nc.vector.tensor_copy(out=hij_sb[:, cs], in_=hij_ps[:, :cw])
                    hji_ps = psE.tile([KDIM, 512], fp32, space="PSUM", tag="hjips")
                    nc.tensor.matmul(out=hji_ps[:, :cw], lhsT=Wp0_sb[:],
                                     rhs=emT[:, :cw], start=True, stop=False)
                    nc.tensor.matmul(out=hji_ps[:, :cw], lhsT=Wp1_sb[:],
                                     rhs=gdt[:, :cw], start=False, stop=False)
                    nc.tensor.matmul(out=hji_ps[:, :cw], lhsT=Wp2_sb[:],
                                     rhs=gst[:, :cw], start=False, stop=True)
                    nc.vector.tensor_copy(out=hji_sb[:, cs], in_=hji_ps[:, :cw])

            # ---------- blocks + scatter ----------
            runs_by_tile = {}
            for (ti, col0, ln, kind, t, lead) in runs:
                runs_by_tile.setdefault(ti, []).append((col0, ln, kind, t, lead))
            for ti in runs_by_tile:
                # stomp-runs (lead>0) first so later runs overwrite their lead
                runs_by_tile[ti].sort(key=lambda r: -r[4])
            with tc.tile_pool(name="psBK", bufs=2, space="PSUM") as psBK:
                for ti in range(n_tiles):
                    bps = psBK.tile([128, I_EDGE], fp32, space="PSUM", tag="bps")
                    nc.vector.tensor_copy(out=bps[:], in_=ztile[:, :I_EDGE])
                    for (col0, ln, kind, t, lead) in runs_by_tile.get(ti, []):
                        sl = slice(ti * 128 + col0 - lead,
                                   ti * 128 + col0 + ln)
                        oap = bps[col0 - lead:col0 + ln, :]
                        tt = slice(t * 169, (t + 1) * 169)
                        if kind == 0:
                            nc.tensor.matmul(out=oap, lhsT=hij_sb[:, sl],
                                             rhs=C1_sb[:, tt], start=True,
                                             stop=False)
                            nc.tensor.matmul(out=oap, lhsT=hji_sb[:, sl],
                                             rhs=C2_sb[:, tt], start=False,
                                             stop=True)
                        elif kind == 1:
                            nc.tensor.matmul(out=oap, lhsT=hji_sb[:, sl],
                                             rhs=C1_sb[:, tt], start=True,
                                             stop=False)
                            nc.tensor.matmul(out=oap, lhsT=hij_sb[:, sl],
                                             rhs=C2_sb[:, tt], start=False,
                                             stop=True)
                        else:
                            nc.tensor.matmul(out=oap, lhsT=hij_sb[:, sl],
                                             rhs=Ccb_sb[:, tt], start=True,
                                             stop=True)
                    blk = blkpool.tile([128, I_EDGE], fp32, tag="blk")
                    nc.vector.tensor_copy(out=blk[:], in_=bps[:])
                    nc.gpsimd.indirect_dma_start(
                        out=fam[:].rearrange("r (cb e) -> (r cb) e", e=I_EDGE),
                        out_offset=bass.IndirectOffsetOnAxis(
                            ap=scat_sb[:, ti:ti + 1], axis=0),
                        in_=blk[:], in_offset=None, compute_op=OP.add)
    nc.finalize()
    return nc


# ---------------- host inputs per core ----------------
def _make_in_maps(inputs, sched):
    src = inputs["edge_index"][0].astype(np.int64)
    dst = inputs["edge_index"][1].astype(np.int64)
    ntypes = inputs["node_types"].astype(np.int64)
    A = _build_adjacency(src, dst)
    n_slots = sched["n_slots"]
    n_tiles = sched["n_tiles"]

    nf = np.ascontiguousarray(inputs["node_feats"], np.float32)
    shared = {
        "nfT": np.ascontiguousarray(nf.T),
        "nf": nf,
        "naT": np.ascontiguousarray(inputs["node_attrs"].T, dtype=np.float32),
        "A": A,
        "Wmsg": np.ascontiguousarray(inputs["W_msg"], np.float32),
        "Wattr": np.ascontiguousarray(inputs["W_attr"], np.float32),
        "W1": np.ascontiguousarray(inputs["W_em"][:F], np.float32),
        "W2": np.ascontiguousarray(inputs["W_em"][F:2 * F], np.float32),
        "W34": np.ascontiguousarray(inputs["W_em"][2 * F:], np.float32),
        "Wp0": np.ascontiguousarray(inputs["W_proj"][:H], np.float32),
        "Wp1": np.ascontiguousarray(inputs["W_proj"][H:H + F], np.float32),
        "Wp2": np.ascontiguousarray(inputs["W_proj"][H + F:], np.float32),
        "WeT": np.ascontiguousarray(
            inputs["W_edge"].transpose(0, 2, 1).reshape(P * I_EDGE, KDIM)),
        "cobF": np.ascontiguousarray(
            inputs["cob_edge"].reshape(P, I_EDGE, I_EDGE)
            .reshape(P * I_EDGE, I_EDGE)),
        "cobFs": np.ascontiguousarray(
            inputs["cob_edge"].transpose(0, 1, 3, 2)
            .reshape(P * I_EDGE, I_EDGE)),
        "WnT": np.ascontiguousarray(
            inputs["W_node"].transpose(0, 2, 1).reshape(T * I_NODE, F)),
        "cobnF": np.ascontiguousarray(
            inputs["cob_node"].reshape(T, I_NODE, I_EDGE)
            .reshape(T * I_NODE, I_EDGE)),
    }
    ef = np.concatenate([inputs["edge_feats"], inputs["edge_attrs"]],
                        axis=1).astype(np.float32)  # [E, 32]

    in_maps = []
    for k in range(NCORES):
        c = sched["cores"][k]
        base = k * NPC
        eid = c["eid"]
        efea = np.zeros((n_slots, 2 * EA), np.float32)
        valid = eid >= 0
        efea[valid] = ef[eid[valid]]
        masks = np.zeros((128, T), np.float32)
        own_t = ntypes[base:base + NPC]
        for tau in range(T):
            masks[:NPC, tau] = (own_t == tau)
        nbgath = np.zeros((128, 1), np.int32)
        nbgath[:NPC, 0] = np.arange(base, base + NPC)
        nbscat = np.full((128, 1), DUMP_SLOT, np.int32)
        nbscat[:NPC, 0] = np.arange(NPC) * N + np.arange(base, base + NPC)
        im = dict(shared)
        im.update({
            "masks": masks,
            "gsrc": np.ascontiguousarray(
                c["gsrc"].reshape(n_tiles, 128).T.astype(np.int32)),
            "gdst": np.ascontiguousarray(
                c["gdst"].reshape(n_tiles, 128).T.astype(np.int32)),
            "scat": np.ascontiguousarray(
                c["scat"].reshape(n_tiles, 128).T.astype(np.int32)),
            "nbgath": nbgath,
            "nbscat": nbscat,
            "efeaT": np.ascontiguousarray(efea.T),
        })
        in_maps.append(im)
    return in_maps


_CACHE = {}


def kernel(**inputs):
    from concourse.bass_utils import run_bass_kernel_spmd

    inputs = {k: np.asarray(v) for k, v in inputs.items()}
    sched = _build_schedule(inputs["edge_index"], inputs["edge_types"])
    key = (sched["n_tiles"], tuple(sched["runs"]))
    if key not in _CACHE:
        _CACHE[key] = _build_bass(sched["n_tiles"], sched["runs"])
    nc = _CACHE[key]
    in_maps = _make_in_maps(inputs, sched)
    res = run_bass_kernel_spmd(nc, in_maps, list(range(NCORES)))
    return assemble(res.results)


def assemble(results):
    shards = []
    for k in range(NCORES):
        fam = results[k]["fam"][:NPC].reshape(NPC, N, D, D)
        shards.append(np.ascontiguousarray(fam.transpose(0, 2, 1, 3))
                      .reshape(NPC * D, N * D))
    return np.concatenate(shards, axis=0)


if __name__ == "__main__":
    import jax
    sys.path.insert(0, "/root/problem")
    import reference
    with jax.default_device(jax.local_devices(backend="cpu")[0]):
        inp = {k: np.asarray(v) for k, v in reference.setup_inputs().items()}
        expected = np.asarray(reference.reference(**inp))
    got = kernel(**inp)
    err = np.abs(got - expected).max()
    rel = err / np.abs(expected).max()
    print("max abs err", err, "rel", rel)
    print("PASS" if rel < 1e-4 else "FAIL")
